# revision 6
# baseline (speedup 1.0000x reference)
"""GAT message-passing kernel for 8 Trainium2 NeuronCores.

Key algebraic property of the reference (faithful torch repeat_interleave
replication): with h = x @ proj_w.T + proj_b  [B, N, H],
    first[b, I, J, c]  = h[b, I, J // (N//H)]
    second[b, I, J, c] = h[b, I, c]
so the pre-mask score collapses to
    scores[b, I, J] = leaky_relu(S1 * h[b, I, J//32] + d[b, I])
with S1 = sum(a_w[0, :H]) and d = h @ a_w[0, H:].  Each row of scores has
only H=32 distinct values (one per 32-column block of J).  Softmax+matmul
then reduce to a masked weighted aggregation that never materializes any
[N, N] tensor in HBM:
    W[b, I, J] = adj[I, J] * exp(leaky(v))[b, I, J//32]
    out[b, I, :] = (W @ h[b]) / rowsum(W)

Sharding: rows I are split 128-per-core across 8 cores (both batches on
every core); x and the tiny weights are replicated.

Device-side critical path design:

1. J-side tiling J = 8*q + tk (q = partition, tk = tile 0..7) makes
   k(J) = J//32 = q//4 constant per q, so the score broadcast over J
   collapses to a partition-level broadcast k -> q, folded into the
   *first* matmul on the host side: wmb = wm @ IND with
   IND[k, q] = (q//4 == k).  One [65x128] x [65x128] matmul per batch
   produces vb[q, I] directly (bias rides as a ones-row in xoT);
   leaky_relu and exp are per-batch ACT ops so batch 0's W tiles start
   as early as possible.  No indicator matmul, no replicate DMA.

2. The adjacency threshold is evaluated on the host in exact fp32
   (alongside the transpose/diagonal-fix input prep): rounding dist to a
   16-bit type before the compare flips adjacency bits on dominant-
   weight neighbors and costs ~16% output error, while a 0/1 mask is
   exact in bf16.  On device W tiles are a plain bf16 tensor_tensor
   multiply (2x DVE mode); eb is doubled in SBUF once (a 125 ns copy)
   so each multiply spans two J-tiles, halving the per-op fixed cost.
   Every stream is bf16 (fp32 PSUM accumulation): half the HBM bytes
   and 4x faster PE matmuls than fp32.

3. Aggregation associativity: out = (W @ x_aug) @ wt_aug with
   x_aug = [x | 1]; GT[c, I] accumulates in a single PSUM bank per batch
   (8 chained matmuls whose lhsT is the natural-layout x_aug tile).
   GT[64, :] = Z (softmax denominator).  PSUM->SBUF spills run on the
   otherwise-idle ACT engine.  One final [65, 33] matmul per batch
   applies wt_aug (bias row + Z column); the divide by Z happens on
   host.

cb/xoT/mask ride the Sync HWDGE ring (the ACT ring's first issue hides
behind the 1.3 us activation-table load, so xa rides ACT).  One combined
output store.  Zero PE transposes, zero GpSimd ops.
"""

import sys

sys.path.insert(0, "/opt/trn_rl_repo")

import numpy as np

B, N, C, H = 2, 1024, 64, 32
P = 128                 # rows per core / partition tile
NCORES = 8
NJ = N // P             # 8 J-tiles of 128
THR = 200000.0
ALPHA = 0.01
H1 = H + 1              # 33: h channels + Z column
C1 = C + 1              # 65: x channels + ones column

_CACHE = {}
LAST_RESULT = None


def _build():
    import concourse.bacc as bacc
    import concourse.tile as tile
    from concourse import mybir

    F32 = mybir.dt.float32
    BF16 = mybir.dt.bfloat16
    Act = mybir.ActivationFunctionType

    nc = bacc.Bacc("TRN2", target_bir_lowering=False)

    # xoT rows 0:64 = x[:, core rows, :].T (both batches); row 64 = ones
    xoT_d = nc.dram_tensor("xoT", (C1, B * P), BF16, kind="ExternalInput")
    # mT[q, 128*t + i] = (dist.T[8q + t, core_row i] < thr), diag forced 1
    mT_d = nc.dram_tensor("mT", (P, NJ * P), BF16, kind="ExternalInput")
    # xa[q, b, 65*t + c] = x_aug[b, 8q + t, c]  (c = 64 -> 1.0)
    xa_d = nc.dram_tensor("xa", (P, B, NJ * C1), BF16, kind="ExternalInput")
    # cb cols 0:128 = wmb_aug (rows 0:65), cols 128:161 = wfin (rows 0:65)
    cb_d = nc.dram_tensor("cb", (P, P + H1), BF16, kind="ExternalInput")
    # un-normalized output + Z column, both batches; host divides + transposes
    out_d = nc.dram_tensor("out", (P, B, H1), F32, kind="ExternalOutput")

    HW = NJ * P // 2  # mask arrives in two halves of 4 J-tiles

    with tile.TileContext(nc) as tc:
        with (
            tc.tile_pool(name="const", bufs=1) as const,
            tc.tile_pool(name="persist", bufs=1) as persist,
            tc.tile_pool(name="work", bufs=2) as work,
            tc.tile_pool(name="psV", bufs=2, space="PSUM") as psV,
            tc.tile_pool(name="psG", bufs=2, space="PSUM") as psG,
            tc.tile_pool(name="psF", bufs=2, space="PSUM") as psF,
        ):
            # ---- input DMAs (two HWDGE rings, first-needed first) ----
            cb = const.tile([P, P + H1], BF16)
            nc.sync.dma_start(out=cb[:], in_=cb_d[:])
            xoT = const.tile([C1, B * P], BF16)
            nc.sync.dma_start(out=xoT[:], in_=xoT_d[:])
            mT = persist.tile([P, NJ * P], BF16)
            nc.sync.dma_start(out=mT[:, 0:HW], in_=mT_d[:, 0:HW])
            xa = persist.tile([P, B, NJ * C1], BF16)
            nc.scalar.dma_start(out=xa[:, 0, :], in_=xa_d[:, 0, :])
            nc.sync.dma_start(out=mT[:, HW:], in_=mT_d[:, HW:])
            nc.scalar.dma_start(out=xa[:, 1, :], in_=xa_d[:, 1, :])

            wmb = cb[0:C1, 0:P]
            wfin = cb[0:C1, P : P + H1]

            # ---- per-batch scores -> eb (doubled in SBUF for wide TTs) ----
            ebx = {}

            def scores(b):
                ps_v = psV.tile([P, P], F32, tag="v", name=f"v{b}")
                nc.tensor.matmul(ps_v[:], wmb, xoT[:, b * P : (b + 1) * P])
                t_sb = work.tile([P, P], BF16, tag="t", name=f"t{b}")
                nc.scalar.activation(t_sb[:], ps_v[:], Act.Prelu, alpha=ALPHA)
                e2 = persist.tile([P, 2 * P], BF16, tag="ebx", name=f"ebx{b}")
                nc.scalar.activation(e2[:, 0:P], t_sb[:], Act.Exp)
                nc.vector.tensor_copy(e2[:, P : 2 * P], e2[:, 0:P])
                ebx[b] = e2

            gts = {}

            def agg(b):
                # W tiles: mask * eb, bf16 2x-mode, two J-tiles per DVE op
                wt = work.tile([P, NJ * P], BF16, tag="wt", name=f"wt{b}")
                ps_g = psG.tile([C1, P], F32, tag="g", name=f"g{b}")
                for tk in range(NJ):
                    if tk % 2 == 0:
                        sl = slice(tk * P, (tk + 2) * P)
                        nc.vector.tensor_mul(wt[:, sl], mT[:, sl], ebx[b][:])
                    nc.tensor.matmul(
                        ps_g[:],
                        xa[:, b, C1 * tk : C1 * tk + C1],
                        wt[:, tk * P : (tk + 1) * P],
                        start=(tk == 0),
                        stop=(tk == NJ - 1),
                    )
                # PSUM -> SBUF spill on the idle ACT engine
                gt = work.tile([C1, P], BF16, tag="gt", name=f"gt{b}")
                nc.scalar.copy(gt[:], ps_g[:])
                gts[b] = gt

            scores(0)
            scores(1)
            agg(0)
            agg(1)

            ot = work.tile([P, B, H1], F32, tag="ot")
            for b in (0, 1):
                ps_f = psF.tile([P, H1], F32, tag="f", name=f"f{b}")
                nc.tensor.matmul(ps_f[:], gts[b][:], wfin)
                nc.vector.tensor_copy(ot[:, b, :], ps_f[:])
            nc.sync.dma_start(out=out_d[:], in_=ot[:])

    nc.finalize()
    return nc


def kernel(x, dist_mat, proj_w, proj_b, a_w, trace=False):
    global LAST_RESULT
    import ml_dtypes
    from concourse.bass_utils import run_bass_kernel_spmd

    BF = ml_dtypes.bfloat16
    x = np.ascontiguousarray(np.asarray(x, dtype=np.float32))
    dist_mat = np.asarray(dist_mat, dtype=np.float32)
    proj_w = np.asarray(proj_w, dtype=np.float32)
    proj_b = np.asarray(proj_b, dtype=np.float32).reshape(H)
    a_w = np.asarray(a_w, dtype=np.float32).reshape(2 * H)

    if "nc" not in _CACHE:
        _CACHE["nc"] = _build()
    nc = _CACHE["nc"]

    # ---- host-side constant folding (all tiny) ----
    a1, a2 = a_w[:H], a_w[H:]
    s1 = np.float32(a1.sum(dtype=np.float32))
    m32 = s1 * np.eye(H, dtype=np.float32) + a2[:, None]  # v = m32.T @ hT
    wta = proj_w.T.astype(np.float32)                     # [C, H]
    wm = wta @ m32                                        # fold h->v projection
    vcol = m32.T @ proj_b                                 # [H]
    # k -> q indicator: IND[k, q] = 1 iff q//4 == k
    ind = np.zeros((H, P), np.float32)
    for k in range(H):
        ind[k, 4 * k : 4 * k + 4] = 1.0
    wmb_aug = np.zeros((C1, P), np.float32)
    wmb_aug[:C] = wm @ ind
    wmb_aug[C] = vcol @ ind                               # bias row (ones in xoT)
    # final projection [x | 1] -> [h | Z]: bias row, Z column
    wfin = np.zeros((C1, H1), np.float32)
    wfin[:C, :H] = wta
    wfin[C, :H] = proj_b
    wfin[C, H] = 1.0
    cb0 = np.zeros((P, P + H1), BF)
    cb0[0:C1, 0:P] = wmb_aug.astype(BF)
    cb0[0:C1, P : P + H1] = wfin.astype(BF)

    # adjacency in exact fp32 on host; 0/1 is exact in bf16
    dist_fixed = dist_mat.copy()
    np.fill_diagonal(dist_fixed, 0.0)  # adj diagonal forced to 1
    maskT = np.ascontiguousarray((dist_fixed.T < THR).astype(BF))  # [N(J), N(I)]

    # token J = 8*q + tk; xa[q, b, 65*tk + c] with trailing 1.0 per token
    xa = np.ones((B, N, C1), np.float32)
    xa[:, :, :C] = x
    xa = np.ascontiguousarray(
        xa.reshape(B, P, NJ * C1).transpose(1, 0, 2).astype(BF)
    )

    xbf = x.astype(BF)
    in_maps = []
    for c in range(NCORES):
        sl = slice(c * P, (c + 1) * P)
        xoT = np.ones((C1, B * P), BF)
        xoT[:C, 0:P] = xbf[0, sl, :].T
        xoT[:C, P : 2 * P] = xbf[1, sl, :].T
        in_maps.append(
            {
                "xa": xa,
                "xoT": xoT,
                "mT": np.ascontiguousarray(maskT[:, sl]).reshape(P, NJ * P),
                "cb": cb0,
            }
        )

    res = run_bass_kernel_spmd(nc, in_maps, core_ids=list(range(NCORES)), trace=trace)
    LAST_RESULT = res
    # out is [P, B, H1] per core -> [B, N, H1]
    full = np.concatenate(
        [res.results[c]["out"].transpose(1, 0, 2) for c in range(NCORES)], axis=1
    )
    return np.ascontiguousarray(full[:, :, :H] / full[:, :, H : H + 1])


# revision 9
# speedup vs baseline: 1.2099x; 1.2099x over previous
"""GAT message-passing kernel for 8 Trainium2 NeuronCores.

Key algebraic property of the reference (faithful torch repeat_interleave
replication): with h = x @ proj_w.T + proj_b  [B, N, H],
    first[b, I, J, c]  = h[b, I, J // (N//H)]
    second[b, I, J, c] = h[b, I, c]
so the pre-mask score collapses to
    scores[b, I, J] = leaky_relu(S1 * h[b, I, J//32] + d[b, I])
with S1 = sum(a_w[0, :H]) and d = h @ a_w[0, H:].  Each row of scores has
only H=32 distinct values (one per 32-column block of J).  Softmax+matmul
then reduce to a masked weighted aggregation that never materializes any
[N, N] tensor in HBM:
    W[b, I, J] = adj[I, J] * exp(leaky(v))[b, I, J//32]
    out[b, I, :] = (W @ h[b]) / rowsum(W)

Sharding: rows I are split 128-per-core across 8 cores (both batches on
every core); x and the tiny weights are replicated.

Device-side critical path design:

1. J-side tiling J = 8*q + tk (q = partition, tk = tile 0..7) makes
   k(J) = J//32 = q//4 constant per q, so the score broadcast over J
   collapses to a partition-level broadcast k -> q, folded into the
   *first* matmul on the host side: wmb = wm @ IND with
   IND[k, q] = (q//4 == k).  One [65x128] x [65x128] matmul per batch
   produces vb[q, I] directly (bias rides as a ones-row in xoT);
   leaky_relu and exp are per-batch ACT ops so batch 0's W tiles start
   as early as possible.  No indicator matmul, no replicate DMA.

2. The adjacency threshold is evaluated on the host in exact fp32
   (alongside the transpose/diagonal-fix input prep): rounding dist to a
   16-bit type before the compare flips adjacency bits on dominant-
   weight neighbors and costs ~16% output error, while a 0/1 mask is
   exact in bf16.  On device W tiles are a plain bf16 tensor_tensor
   multiply (2x DVE mode); eb is doubled in SBUF once (a 125 ns copy)
   so each multiply spans two J-tiles, halving the per-op fixed cost.
   Every stream is bf16 (fp32 PSUM accumulation): half the HBM bytes
   and 4x faster PE matmuls than fp32.

3. Aggregation associativity: out = (W @ x_aug) @ wt_aug with
   x_aug = [x | 1]; GT[c, I] accumulates in a single PSUM bank per batch
   (8 chained matmuls whose lhsT is the natural-layout x_aug tile).
   GT[64, :] = Z (softmax denominator).  PSUM->SBUF spills run on the
   otherwise-idle ACT engine.  One final [65, 33] matmul per batch
   applies wt_aug (bias row + Z column); the divide by Z happens on
   host.

cb/xoT/mask ride the Sync HWDGE ring (the ACT ring's first issue hides
behind the 1.3 us activation-table load, so xa rides ACT).  One combined
output store.  Zero PE transposes, zero GpSimd ops.
"""

import sys

sys.path.insert(0, "/opt/trn_rl_repo")

import numpy as np

B, N, C, H = 2, 1024, 64, 32
P = 128                 # rows per core / partition tile
NCORES = 8
NJ = N // P             # 8 J-tiles of 128
THR = 200000.0
ALPHA = 0.01
H1 = H + 1              # 33: h channels + Z column
C1 = C + 1              # 65: x channels + ones column

_CACHE = {}
LAST_RESULT = None


def _build():
    import concourse.bacc as bacc
    import concourse.tile as tile
    from concourse import mybir

    F32 = mybir.dt.float32
    BF16 = mybir.dt.bfloat16
    Act = mybir.ActivationFunctionType

    nc = bacc.Bacc("TRN2", target_bir_lowering=False)

    # xoT rows 0:64 = x[:, core rows, :].T (both batches); row 64 = ones
    xoT_d = nc.dram_tensor("xoT", (C1, B * P), BF16, kind="ExternalInput")
    # mT[q, 128*t + i] = (dist.T[8q + t, core_row i] < thr), diag forced 1
    mT_d = nc.dram_tensor("mT", (P, NJ * P), BF16, kind="ExternalInput")
    # xa[q, b, 65*t + c] = x_aug[b, 8q + t, c]  (c = 64 -> 1.0)
    xa_d = nc.dram_tensor("xa", (P, B, NJ * C1), BF16, kind="ExternalInput")
    # cb cols 0:128 = wmb_aug (rows 0:65), cols 128:161 = wfin (rows 0:65)
    cb_d = nc.dram_tensor("cb", (P, P + H1), BF16, kind="ExternalInput")
    # un-normalized output + Z column, both batches; host divides + transposes
    out_d = nc.dram_tensor("out", (P, B, H1), F32, kind="ExternalOutput")

    HW = NJ * P // 2  # mask arrives in two halves of 4 J-tiles

    with tile.TileContext(nc) as tc:
        with (
            tc.tile_pool(name="const", bufs=1) as const,
            tc.tile_pool(name="persist", bufs=1) as persist,
            tc.tile_pool(name="work", bufs=2) as work,
            tc.tile_pool(name="psV", bufs=2, space="PSUM") as psV,
            tc.tile_pool(name="psG", bufs=2, space="PSUM") as psG,
            tc.tile_pool(name="psF", bufs=2, space="PSUM") as psF,
        ):
            # ---- input DMAs (two HWDGE rings, first-needed first) ----
            xoT = const.tile([C1, B * P], BF16)
            nc.sync.dma_start(out=xoT[:], in_=xoT_d[:])
            cb = const.tile([P, P + H1], BF16)
            nc.scalar.dma_start(out=cb[:], in_=cb_d[:])
            mT = persist.tile([P, NJ * P], BF16)
            nc.sync.dma_start(out=mT[:, 0:HW], in_=mT_d[:, 0:HW])
            xa = persist.tile([P, B, NJ * C1], BF16)
            nc.scalar.dma_start(out=xa[:, 0, :], in_=xa_d[:, 0, :])
            nc.sync.dma_start(out=mT[:, HW:], in_=mT_d[:, HW:])
            nc.scalar.dma_start(out=xa[:, 1, :], in_=xa_d[:, 1, :])

            wmb = cb[0:C1, 0:P]
            wfin = cb[0:C1, P : P + H1]

            # ---- per-batch scores -> eb (doubled in SBUF for wide TTs) ----
            ebx = {}

            def scores(b):
                ps_v = psV.tile([P, P], F32, tag="v", name=f"v{b}")
                nc.tensor.matmul(ps_v[:], wmb, xoT[:, b * P : (b + 1) * P])
                t_sb = work.tile([P, P], BF16, tag="t", name=f"t{b}")
                nc.scalar.activation(t_sb[:], ps_v[:], Act.Prelu, alpha=ALPHA)
                e2 = persist.tile([P, 2 * P], BF16, tag=f"ebx{b}", name=f"ebx{b}")
                nc.scalar.activation(e2[:, 0:P], t_sb[:], Act.Exp)
                nc.vector.tensor_copy(e2[:, P : 2 * P], e2[:, 0:P])
                ebx[b] = e2

            gts = {}

            def agg(b):
                # W tiles: mask * eb, bf16 2x-mode, two J-tiles per DVE op
                wt = work.tile([P, NJ * P], BF16, tag="wt", name=f"wt{b}")
                ps_g = psG.tile([C1, P], F32, tag="g", name=f"g{b}")
                for tk in range(NJ):
                    if tk % 2 == 0:
                        sl = slice(tk * P, (tk + 2) * P)
                        nc.vector.tensor_mul(wt[:, sl], mT[:, sl], ebx[b][:])
                    nc.tensor.matmul(
                        ps_g[:],
                        xa[:, b, C1 * tk : C1 * tk + C1],
                        wt[:, tk * P : (tk + 1) * P],
                        start=(tk == 0),
                        stop=(tk == NJ - 1),
                    )
                # PSUM -> SBUF spill: ACT for b0 (hidden under b1 work),
                # DVE for b1 (faster; on the critical epilogue path)
                gt = work.tile([C1, P], BF16, tag="gt", name=f"gt{b}")
                if b == 0:
                    nc.scalar.copy(gt[:], ps_g[:])
                else:
                    nc.vector.tensor_copy(gt[:], ps_g[:])
                gts[b] = gt

            scores(0)
            scores(1)
            agg(0)
            agg(1)

            ot = work.tile([P, B, H1], F32, tag="ot")
            for b in (0, 1):
                ps_f = psF.tile([P, H1], F32, tag="f", name=f"f{b}")
                nc.tensor.matmul(ps_f[:], gts[b][:], wfin)
                nc.vector.tensor_copy(ot[:, b, :], ps_f[:])
            nc.sync.dma_start(out=out_d[:], in_=ot[:])

    nc.finalize()
    return nc


def kernel(x, dist_mat, proj_w, proj_b, a_w, trace=False):
    global LAST_RESULT
    import ml_dtypes
    from concourse.bass_utils import run_bass_kernel_spmd

    BF = ml_dtypes.bfloat16
    x = np.ascontiguousarray(np.asarray(x, dtype=np.float32))
    dist_mat = np.asarray(dist_mat, dtype=np.float32)
    proj_w = np.asarray(proj_w, dtype=np.float32)
    proj_b = np.asarray(proj_b, dtype=np.float32).reshape(H)
    a_w = np.asarray(a_w, dtype=np.float32).reshape(2 * H)

    if "nc" not in _CACHE:
        _CACHE["nc"] = _build()
    nc = _CACHE["nc"]

    # ---- host-side constant folding (all tiny) ----
    a1, a2 = a_w[:H], a_w[H:]
    s1 = np.float32(a1.sum(dtype=np.float32))
    m32 = s1 * np.eye(H, dtype=np.float32) + a2[:, None]  # v = m32.T @ hT
    wta = proj_w.T.astype(np.float32)                     # [C, H]
    wm = wta @ m32                                        # fold h->v projection
    vcol = m32.T @ proj_b                                 # [H]
    # k -> q indicator: IND[k, q] = 1 iff q//4 == k
    ind = np.zeros((H, P), np.float32)
    for k in range(H):
        ind[k, 4 * k : 4 * k + 4] = 1.0
    wmb_aug = np.zeros((C1, P), np.float32)
    wmb_aug[:C] = wm @ ind
    wmb_aug[C] = vcol @ ind                               # bias row (ones in xoT)
    # final projection [x | 1] -> [h | Z]: bias row, Z column
    wfin = np.zeros((C1, H1), np.float32)
    wfin[:C, :H] = wta
    wfin[C, :H] = proj_b
    wfin[C, H] = 1.0
    cb0 = np.zeros((P, P + H1), BF)
    cb0[0:C1, 0:P] = wmb_aug.astype(BF)
    cb0[0:C1, P : P + H1] = wfin.astype(BF)

    # adjacency in exact fp32 on host; 0/1 is exact in bf16
    dist_fixed = dist_mat.copy()
    np.fill_diagonal(dist_fixed, 0.0)  # adj diagonal forced to 1
    maskT = np.ascontiguousarray((dist_fixed.T < THR).astype(BF))  # [N(J), N(I)]

    # token J = 8*q + tk; xa[q, b, 65*tk + c] with trailing 1.0 per token
    xa = np.ones((B, N, C1), np.float32)
    xa[:, :, :C] = x
    xa = np.ascontiguousarray(
        xa.reshape(B, P, NJ * C1).transpose(1, 0, 2).astype(BF)
    )

    xbf = x.astype(BF)
    in_maps = []
    for c in range(NCORES):
        sl = slice(c * P, (c + 1) * P)
        xoT = np.ones((C1, B * P), BF)
        xoT[:C, 0:P] = xbf[0, sl, :].T
        xoT[:C, P : 2 * P] = xbf[1, sl, :].T
        in_maps.append(
            {
                "xa": xa,
                "xoT": xoT,
                "mT": np.ascontiguousarray(maskT[:, sl]).reshape(P, NJ * P),
                "cb": cb0,
            }
        )

    res = run_bass_kernel_spmd(nc, in_maps, core_ids=list(range(NCORES)), trace=trace)
    LAST_RESULT = res
    # out is [P, B, H1] per core -> [B, N, H1]
    full = np.concatenate(
        [res.results[c]["out"].transpose(1, 0, 2) for c in range(NCORES)], axis=1
    )
    return np.ascontiguousarray(full[:, :, :H] / full[:, :, H : H + 1])


# revision 10
# speedup vs baseline: 1.2281x; 1.0150x over previous
"""GAT message-passing kernel for 8 Trainium2 NeuronCores.

Key algebraic property of the reference (faithful torch repeat_interleave
replication): with h = x @ proj_w.T + proj_b  [B, N, H],
    first[b, I, J, c]  = h[b, I, J // (N//H)]
    second[b, I, J, c] = h[b, I, c]
so the pre-mask score collapses to
    scores[b, I, J] = leaky_relu(S1 * h[b, I, J//32] + d[b, I])
with S1 = sum(a_w[0, :H]) and d = h @ a_w[0, H:].  Each row of scores has
only H=32 distinct values (one per 32-column block of J).  Softmax+matmul
then reduce to a masked weighted aggregation that never materializes any
[N, N] tensor in HBM:
    W[b, I, J] = adj[I, J] * exp(leaky(v))[b, I, J//32]
    out[b, I, :] = (W @ h[b]) / rowsum(W)

Sharding: rows I are split 128-per-core across 8 cores (both batches on
every core); x and the tiny weights are replicated.

Device-side critical path design:

1. J-side tiling J = 8*q + tk (q = partition, tk = tile 0..7) makes
   k(J) = J//32 = q//4 constant per q, so the score broadcast over J
   collapses to a partition-level broadcast k -> q, folded into the
   *first* matmul on the host side: wmb = wm @ IND with
   IND[k, q] = (q//4 == k).  One [65x128] x [65x256] matmul produces
   vb[q, I] for both batches (bias rides as a ones-row next to xoT in
   the same first DMA); leaky_relu and exp are two full-width ACT ops.
   No indicator matmul, no replicate DMA.

2. The adjacency threshold is evaluated on the host in exact fp32
   (alongside the transpose/diagonal-fix input prep): rounding dist to a
   16-bit type before the compare flips adjacency bits on dominant-
   weight neighbors and costs ~16% output error, while a 0/1 mask is
   exact in bf16.  The host also ships the mask *doubled* along the
   row axis ([tk, b*128+i] layout) so one bf16 2x-mode tensor_tensor
   per J-tile builds the W tiles of BOTH batches against the natural
   [128, 256] eb tile - 8 DVE ops total, no broadcast tricks, and
   batch 1 never waits for a second TT phase.  The mask streams in
   four quarter-DMAs sized to arrive just-in-time under the TT cadence.

3. Aggregation associativity: out = (W @ x_aug) @ wt_aug with
   x_aug = [x | 1]; GT[c, I] accumulates per batch in its own PSUM bank
   (two interleaved 8-matmul chains whose lhsT is the natural-layout
   x_aug tile).  GT[64, :] = Z (softmax denominator).  The two GT spills
   land side by side in one [65, 256] tile (ACT engine for batch 0,
   DVE for batch 1), so a single [33, 256] matmul applies wt_aug (bias
   row + Z column) for both batches; the divide by Z happens on host.

Every stream is bf16 (fp32 PSUM accumulation): half the HBM bytes and
4x faster PE matmuls than fp32.  One combined output store; zero PE
transposes; zero GpSimd ops.
"""

import sys

sys.path.insert(0, "/opt/trn_rl_repo")

import numpy as np

B, N, C, H = 2, 1024, 64, 32
P = 128                 # rows per core / partition tile
NCORES = 8
NJ = N // P             # 8 J-tiles of 128
THR = 200000.0
ALPHA = 0.01
H1 = H + 1              # 33: h channels + Z column
C1 = C + 1              # 65: x channels + ones column
WXW = P + B * P + H1    # 417: wmb | xoT | wfin columns

_CACHE = {}
LAST_RESULT = None


def _build():
    import concourse.bacc as bacc
    import concourse.tile as tile
    from concourse import mybir

    F32 = mybir.dt.float32
    BF16 = mybir.dt.bfloat16
    Act = mybir.ActivationFunctionType

    nc = bacc.Bacc("TRN2", target_bir_lowering=False)

    # wx = [wmb_aug | xoT | wfin]; xoT rows 0:64 = x.T slices, row 64 = ones
    wx_d = nc.dram_tensor("wx", (C1, WXW), BF16, kind="ExternalInput")
    # m2[q, 256*t + 128*b + i] = (dist.T[8q + t, core_row i] < thr), diag 1
    m2_d = nc.dram_tensor("m2", (P, NJ * B * P), BF16, kind="ExternalInput")
    # xa[q, b, 65*t + c] = x_aug[b, 8q + t, c]  (c = 64 -> 1.0)
    xa_d = nc.dram_tensor("xa", (P, B, NJ * C1), BF16, kind="ExternalInput")
    # un-normalized [h | Z] output, [33, b*128+i]; host divides + transposes
    out_d = nc.dram_tensor("out", (H1, B * P), F32, kind="ExternalOutput")

    QW = NJ * B * P // 4  # mask quarter: 2 J-tiles x both batches

    with tile.TileContext(nc) as tc:
        with (
            tc.tile_pool(name="const", bufs=1) as const,
            tc.tile_pool(name="persist", bufs=1) as persist,
            tc.tile_pool(name="work", bufs=2) as work,
            tc.tile_pool(name="psV", bufs=1, space="PSUM") as psV,
            tc.tile_pool(name="psG", bufs=2, space="PSUM") as psG,
            tc.tile_pool(name="psF", bufs=1, space="PSUM") as psF,
        ):
            # ---- input DMAs: weights+x.T first, mask quarters JIT ----
            wx = const.tile([C1, WXW], BF16)
            nc.sync.dma_start(out=wx[:], in_=wx_d[:])
            xa = persist.tile([P, B, NJ * C1], BF16)
            nc.scalar.dma_start(out=xa[:, 0, :], in_=xa_d[:, 0, :])
            m2 = persist.tile([P, NJ * B * P], BF16)
            for qq in range(4):
                nc.sync.dma_start(
                    out=m2[:, qq * QW : (qq + 1) * QW],
                    in_=m2_d[:, qq * QW : (qq + 1) * QW],
                )
            nc.scalar.dma_start(out=xa[:, 1, :], in_=xa_d[:, 1, :])

            wmb = wx[0:C1, 0:P]
            xoT = wx[0:C1, P : P + B * P]
            wfin = wx[0:C1, P + B * P : WXW]

            # ---- scores -> eb for both batches: [128, 256] ----
            ps_v = psV.tile([P, B * P], F32)
            nc.tensor.matmul(ps_v[:], wmb, xoT)
            t_sb = work.tile([P, B * P], BF16, tag="t")
            nc.scalar.activation(t_sb[:], ps_v[:], Act.Prelu, alpha=ALPHA)
            eb = persist.tile([P, B * P], BF16)
            nc.scalar.activation(eb[:], t_sb[:], Act.Exp)

            # ---- masked weights + aggregation, both batches per J-tile ----
            wt = persist.tile([P, NJ * B * P], BF16)
            ps_g = {
                b: psG.tile([C1, P], F32, tag="g", name=f"g{b}") for b in (0, 1)
            }
            BP = B * P
            for tk in range(NJ):
                sl = slice(tk * BP, (tk + 1) * BP)
                nc.vector.tensor_mul(wt[:, sl], m2[:, sl], eb[:])
                for b in (0, 1):
                    nc.tensor.matmul(
                        ps_g[b][:],
                        xa[:, b, C1 * tk : C1 * tk + C1],
                        wt[:, tk * BP + b * P : tk * BP + (b + 1) * P],
                        start=(tk == 0),
                        stop=(tk == NJ - 1),
                        skip_group_check=True,
                    )

            # ---- finalize both batches in one [33, 256] matmul ----
            gtb = work.tile([C1, B * P], BF16, tag="gtb")
            nc.scalar.copy(gtb[:, 0:P], ps_g[0][:])        # ACT: hidden
            nc.vector.tensor_copy(gtb[:, P : B * P], ps_g[1][:])  # DVE: fast
            ps_f = psF.tile([H1, B * P], F32)
            nc.tensor.matmul(ps_f[:], wfin, gtb[:])
            ot = work.tile([H1, B * P], F32, tag="ot")
            nc.vector.tensor_copy(ot[:], ps_f[:])
            nc.sync.dma_start(out=out_d[:], in_=ot[:])

    nc.finalize()
    return nc


def kernel(x, dist_mat, proj_w, proj_b, a_w, trace=False):
    global LAST_RESULT
    import ml_dtypes
    from concourse.bass_utils import run_bass_kernel_spmd

    BF = ml_dtypes.bfloat16
    x = np.ascontiguousarray(np.asarray(x, dtype=np.float32))
    dist_mat = np.asarray(dist_mat, dtype=np.float32)
    proj_w = np.asarray(proj_w, dtype=np.float32)
    proj_b = np.asarray(proj_b, dtype=np.float32).reshape(H)
    a_w = np.asarray(a_w, dtype=np.float32).reshape(2 * H)

    if "nc" not in _CACHE:
        _CACHE["nc"] = _build()
    nc = _CACHE["nc"]

    # ---- host-side constant folding (all tiny) ----
    a1, a2 = a_w[:H], a_w[H:]
    s1 = np.float32(a1.sum(dtype=np.float32))
    m32 = s1 * np.eye(H, dtype=np.float32) + a2[:, None]  # v = m32.T @ hT
    wta = proj_w.T.astype(np.float32)                     # [C, H]
    wm = wta @ m32                                        # fold h->v projection
    vcol = m32.T @ proj_b                                 # [H]
    # k -> q indicator: IND[k, q] = 1 iff q//4 == k
    ind = np.zeros((H, P), np.float32)
    for k in range(H):
        ind[k, 4 * k : 4 * k + 4] = 1.0
    wmb_aug = np.zeros((C1, P), np.float32)
    wmb_aug[:C] = wm @ ind
    wmb_aug[C] = vcol @ ind                               # bias row (ones in xoT)
    # final projection [x | 1] -> [h | Z]: bias row, Z column
    wfin = np.zeros((C1, H1), np.float32)
    wfin[:C, :H] = wta
    wfin[C, :H] = proj_b
    wfin[C, H] = 1.0

    # adjacency in exact fp32 on host; 0/1 is exact in bf16; doubled for
    # the both-batches-per-op W-tile multiply
    dist_fixed = dist_mat.copy()
    np.fill_diagonal(dist_fixed, 0.0)  # adj diagonal forced to 1
    maskT = (dist_fixed.T < THR).astype(BF)               # [N(J), N(I)]

    # token J = 8*q + tk; xa[q, b, 65*tk + c] with trailing 1.0 per token
    xa = np.ones((B, N, C1), np.float32)
    xa[:, :, :C] = x
    xa = np.ascontiguousarray(
        xa.reshape(B, P, NJ * C1).transpose(1, 0, 2).astype(BF)
    )

    xbf = x.astype(BF)
    in_maps = []
    for c in range(NCORES):
        sl = slice(c * P, (c + 1) * P)
        wx = np.ones((C1, WXW), BF)
        wx[:, 0:P] = wmb_aug.astype(BF)
        wx[:C, P : P + P] = xbf[0, sl, :].T
        wx[:C, P + P : P + 2 * P] = xbf[1, sl, :].T
        wx[:, P + B * P : WXW] = wfin.astype(BF)
        mc = maskT[:, sl].reshape(P, NJ, P)
        m2 = np.ascontiguousarray(
            np.concatenate([mc, mc], axis=2).reshape(P, NJ * B * P)
        )
        in_maps.append({"xa": xa, "wx": wx, "m2": m2})

    res = run_bass_kernel_spmd(nc, in_maps, core_ids=list(range(NCORES)), trace=trace)
    LAST_RESULT = res
    # out is [H1, B*P] per core -> [B, N, H1]
    full = np.concatenate(
        [
            res.results[c]["out"].reshape(H1, B, P).transpose(1, 2, 0)
            for c in range(NCORES)
        ],
        axis=1,
    )
    return np.ascontiguousarray(full[:, :, :H] / full[:, :, H : H + 1])
